# revision 31
# baseline (speedup 1.0000x reference)
"""CrossModalFusion Trainium2 kernel (weight-folded G-route).

Reference computation (per batch b):
    q = rgb @ Wq + bq                 [S, H]
    k = pose @ Wk + bk                [S, H]
    v = pose @ Wv + bv                [S, H]
    attn = softmax(q @ k.T / sqrt(H)) [S, S]
    out  = attn @ v                   [S, H]
    proj = out @ Wp + bp              [S, D]
    x = rgb + gate * proj
    fused = LayerNorm(x) * gamma + beta

Algebraic restructure (weights folded on the HOST, once):
    X   = (Wk @ Wq.T) / sqrt(H)   [D, D]   so scoresT = (pose X^T?) ... precisely:
          scoresT[sk,sq] = sum_d' uT[d',sk] * rgbT[d',sq],  uT = X.T @ poseT
    c_k = (Wk @ bq) / sqrt(H)     folded as column D of X with a ones-column
          appended to rgb (terms depending only on sq cancel in softmax)
    VWp = Wv @ Wp                 [D, D]   proj = (attn @ pose) @ VWp
    bpg = gate * (bp + bv @ Wp)   added to rgb once per row tile
    colsum(attn) comes FREE from a ones-column in pose (partition 32 of the
          last d-chunk of wT = attn @ pose_aug)

Device work per batch (all matmuls, no PE transposes -- pose/rgb are
transposed by the DMA X-bar in fp16):
    uT[d',sk]  = X-chunks.T @ poseT        (64 MM of N=512)
    per 512-col query block:
      scoresT  = uT-chunks.T @ rgbT        (64 MM)  -> exp on ACT -> attnT fp16
      wT       = pose_aug-chunks.T @ attnT (64 MM)  [row 32 of chunk3 = colsum]
      proj     = wT-chunks.T @ VWp         (16 MM of N=400)
      residual + LayerNorm on DVE/ACT, store

Sharding: pure data-parallel over batch B=32 across 8 cores (4 each).
"""

import numpy as np

B, S, D, H = 32, 2048, 400, 512
DP = 512                 # padded feature dim (multiple of 128)
N_CORES = 8
B_LOC = B // N_CORES
LN_EPS = 1e-5
P = 128
QBLK = 512
ONES_COL = 384           # pose_aug ones column -> csum at partition 0 of chunk3
                         # (d-columns 384:400 shift up by one to 385:401)
CK_COL = 400             # X column holding Wk@bq; rgb_aug ones column

NDCH = DP // P           # 4 d-chunks


def build_nc(b_loc=B_LOC, s=S):
    import concourse.bass as bass
    import concourse.mybir as mybir
    import concourse.tile as tile
    from concourse import bacc

    def bcast(ap1d, p=P):
        return bass.AP(tensor=ap1d.tensor, offset=ap1d.offset,
                       ap=[[0, p]] + list(ap1d.ap))

    f32 = mybir.dt.float32
    f32r = mybir.dt.float32r
    f16 = mybir.dt.float16
    AF = mybir.ActivationFunctionType

    nt = s // P              # seq tiles (16)
    nqb = s // QBLK          # query blocks (4)
    tpb = QBLK // P          # row tiles per block (4)
    nnb = s // QBLK          # 512-wide column splits of s (4)

    nc = bacc.Bacc("TRN2", target_bir_lowering=False, debug=False,
                   num_swdge_queues=4)

    rgb = nc.dram_tensor("rgb", [b_loc, s, D], f32, kind="ExternalInput").ap()
    rgbpT = nc.dram_tensor("rgbpT", [b_loc, DP, s], f16, kind="ExternalInput").ap()
    posep = nc.dram_tensor("posep", [b_loc, s, DP], f16, kind="ExternalInput").ap()
    posepT = nc.dram_tensor("posepT", [b_loc, DP, s], f16, kind="ExternalInput").ap()
    xw = nc.dram_tensor("xw", [DP, DP], f16, kind="ExternalInput").ap()
    vwp = nc.dram_tensor("vwp", [DP, D], f32, kind="ExternalInput").ap()
    bpg = nc.dram_tensor("bpg", [D], f32, kind="ExternalInput").ap()
    gamma = nc.dram_tensor("ln_gamma", [D], f32, kind="ExternalInput").ap()
    beta = nc.dram_tensor("ln_beta", [D], f32, kind="ExternalInput").ap()
    gate = nc.dram_tensor("gate", [1], f32, kind="ExternalInput").ap()
    out = nc.dram_tensor("out", [b_loc, s, D], f32, kind="ExternalOutput").ap()

    from contextlib import ExitStack

    with tile.TileContext(nc) as tc:
        with ExitStack() as ctx:
            pool = lambda **kw: ctx.enter_context(tc.tile_pool(**kw))
            const = pool(name="const", bufs=1)
            wpool = pool(name="wpool", bufs=1)
            wstage = pool(name="wstage", bufs=1)
            ppool = pool(name="ppool", bufs=2)        # pose_aug natural (fp16)
            ptp = pool(name="ptp", bufs=2)            # poseT (fp16)
            rtp = pool(name="rtp", bufs=2)            # rgbT (fp16)
            utp = pool(name="utp", bufs=1)            # uT (fp16)
            atp = pool(name="atp", bufs=1)            # attnT (fp16)
            wtp = pool(name="wtp", bufs=2)            # wT (f32r)
            rpool = pool(name="rpool", bufs=2 * tpb)  # rgb raw (f32)
            ypool = pool(name="ypool", bufs=6)
            small = pool(name="small", bufs=6)
            ps_sc = pool(name="ps_sc", bufs=3, space="PSUM")
            ps_mm = pool(name="ps_mm", bufs=2, space="PSUM")
            ps_pj = pool(name="ps_pj", bufs=2, space="PSUM")
            ps_t1 = pool(name="ps_t1", bufs=1, space="PSUM")

            # ---- constants ----
            ones11 = const.tile([1, 1], f32)
            nc.vector.memset(ones11, 1.0)
            eps_sb = const.tile([P, 1], f32)
            nc.vector.memset(eps_sb, LN_EPS)
            warm_in = const.tile([P, P], f16)
            nc.vector.memset(warm_in, 0.0)

            def emit_pose_dmas(b):
                """Issue batch b's input DMAs, sliced so batch-0 compute can
                chase the transfers: poseT nb-slices first (uT), rgbT
                qb-slices on the scalar ring (scores), pose naturals last
                (wT needs them latest)."""
                pT = ptp.tile([P, NDCH, s], f16, tag="poseT")
                rT = rtp.tile([P, NDCH, s], f16, tag="rgbT")
                for nb in range(nnb):
                    for c in range(NDCH):
                        sl = slice(nb * QBLK, (nb + 1) * QBLK)
                        nc.sync.dma_start(
                            out=pT[:, c, sl],
                            in_=posepT[b, c * P:(c + 1) * P, sl])
                for nb in range(nnb):
                    for c in range(NDCH):
                        sl = slice(nb * QBLK, (nb + 1) * QBLK)
                        nc.sync.dma_start(
                            out=rT[:, c, sl],
                            in_=rgbpT[b, c * P:(c + 1) * P, sl])
                po = ppool.tile([P, nt, DP], f16, tag="pose")
                for t in range(nt):
                    nc.gpsimd.dma_start(
                        out=po[:, t, :], in_=posep[b, t * P:(t + 1) * P, :])
                return po, pT, rT

            def emit_rgb_raw(b, qb):
                """Block qb's residual rgb rows (f32) + bpg pre-add."""
                tiles = []
                for j in range(tpb):
                    r0 = qb * QBLK + j * P
                    rr = rpool.tile([P, D], f32, tag="rraw")
                    nc.scalar.dma_start(out=rr, in_=rgb[b, r0:r0 + P, :])
                    tiles.append(rr)
                return tiles

            def emit_ut(pT):
                """uT[d',sk] = X-chunks.T @ poseT (fp16 out)."""
                uT = utp.tile([P, NDCH, s], f16, tag="uT")
                for nb in range(nnb):
                    for m in range(NDCH):
                        ps = ps_mm.tile([P, QBLK], f32, tag="mm")
                        for c in range(NDCH):
                            nc.tensor.matmul(
                                ps,
                                xw_sb[:, c, m * P:(m + 1) * P],
                                pT[:, c, nb * QBLK:(nb + 1) * QBLK],
                                start=(c == 0), stop=(c == NDCH - 1),
                            )
                        nc.vector.tensor_copy(
                            out=uT[:, m, nb * QBLK:(nb + 1) * QBLK], in_=ps)
                return uT

            def emit_scores(uT, rT, qb):
                """scoresT -> exp -> attnT (fp16, unnormalized)."""
                attnT = atp.tile([P, nt, QBLK], f16, tag="attnT")
                for t in range(nt):
                    ps = ps_sc.tile([P, QBLK], f32, tag="sc")
                    for c in range(NDCH):
                        nc.tensor.matmul(
                            ps,
                            uT[:, c, t * P:(t + 1) * P],
                            rT[:, c, qb * QBLK:(qb + 1) * QBLK],
                            start=(c == 0), stop=(c == NDCH - 1),
                        )
                    nc.scalar.activation(
                        out=attnT[:, t, :], in_=ps, func=AF.Exp)
                return attnT

            def emit_wt(po, attnT):
                """wT = pose_aug-chunks.T @ attnT; chunk3 row 0 = colsum.

                PSUM copies on ACT: during the wT phase ACT is idle (exps ran
                during scores), while the DVE FIFO holds the previous block's
                LayerNorm chain -- DVE copies here would stall the PE on PSUM
                bank recycling."""
                wT = wtp.tile([P, NDCH, QBLK], f32r, tag="wT")
                csum = wtp.tile([1, QBLK], f32, tag="csum")
                for m in range(NDCH):
                    ps = ps_mm.tile([P, QBLK], f32, tag="mm")
                    for t in range(nt):
                        nc.tensor.matmul(
                            ps,
                            po[:, t, m * P:(m + 1) * P],
                            attnT[:, t, :],
                            start=(t == 0), stop=(t == nt - 1),
                        )
                    nc.scalar.copy(out=wT[:, m, :], in_=ps)
                # row 0 of chunk3 = colsum; f32 copy for the f32 scatter matmul
                nc.scalar.copy(out=csum, in_=wT[0:1, 3, :])
                return wT, csum

            def emit_proj(b, qb, wT, csum, rgb_raw):
                """proj + gated residual (phase 1), then LayerNorm + store
                (phase 2).  Phased so the PSUM-freeing STT ops and the next
                block's work never queue behind the LN chain in the DVE
                FIFO."""
                q0 = qb * QBLK
                xs = []
                for j in range(tpb):
                    # csum chunk -> partitions via rank-1 matmul
                    pst = ps_t1.tile([P, 1], f32, tag="t1")
                    nc.tensor.matmul(
                        pst, csum[0:1, j * P:(j + 1) * P], ones11,
                        start=True, stop=True,
                    )
                    rec = small.tile([P, 1], f32, tag="rec")
                    nc.vector.reciprocal(out=rec, in_=pst)
                    gr = small.tile([P, 1], f32, tag="gr")
                    nc.vector.tensor_mul(out=gr, in0=rec, in1=gate_sb)

                    psp = ps_pj.tile([P, D], f32, tag="pj")
                    for c in range(NDCH):
                        # chunk3: K=17 includes csum partition 0 (vwp row is 0)
                        kc = P if c < 3 else (D - 3 * P + 1)
                        nc.tensor.matmul(
                            psp,
                            wT[:kc, c, j * P:(j + 1) * P],
                            vwp_sb[:kc, c, :],
                            start=(c == 0), stop=(c == NDCH - 1),
                        )
                    # x = gr * proj + (rgb + bpg)
                    x = ypool.tile([P, D], f32, tag="x")
                    nc.vector.scalar_tensor_tensor(
                        out=x, in0=psp, scalar=gr, in1=rgb_raw[j],
                        op0=mybir.AluOpType.mult, op1=mybir.AluOpType.add,
                    )
                    xs.append(x)
                for j, x in enumerate(xs):
                    stats = small.tile([P, 6], f32, tag="stats")
                    nc.vector.bn_stats(out=stats, in_=x)
                    mv = small.tile([P, 2], f32, tag="mv")
                    nc.vector.bn_aggr(out=mv, in_=stats)
                    sd = small.tile([P, 1], f32, tag="sd")
                    nc.scalar.activation(
                        out=sd, in_=mv[:, 1:2], func=AF.Sqrt, bias=eps_sb)
                    rstd = small.tile([P, 1], f32, tag="rstd")
                    nc.vector.reciprocal(out=rstd, in_=sd)
                    nc.vector.tensor_scalar(
                        out=x, in0=x, scalar1=mv[:, 0:1], scalar2=rstd,
                        op0=mybir.AluOpType.subtract, op1=mybir.AluOpType.mult,
                    )
                    nc.vector.tensor_mul(out=x, in0=x, in1=gamma_bc)
                    nc.vector.tensor_add(out=x, in0=x, in1=beta_bc)
                    nc.scalar.dma_start(
                        out=out[b, q0 + j * P:q0 + (j + 1) * P, :], in_=x)

            # ---- batch 0 input DMAs first (overlap with weight loads) ----
            pose_state = emit_pose_dmas(0)

            # ---- HAM warmup: keep PE busy while DMAs land ----
            for i in range(70):
                wps = ps_sc.tile([P, QBLK], f32, tag="sc")
                nc.tensor.matmul(
                    wps[:, :P], warm_in, warm_in, start=True, stop=True)

            # ---- weights ----
            xw_sb = wpool.tile([P, NDCH, DP], f16)
            for c in range(NDCH):
                nc.gpsimd.dma_start(
                    out=xw_sb[:, c, :], in_=xw[c * P:(c + 1) * P, :])
            vst = wstage.tile([P, NDCH, D], f32, tag="vst")
            vwp_sb = wpool.tile([P, NDCH, D], f32r)
            for c in range(NDCH):
                nc.gpsimd.dma_start(
                    out=vst[:, c, :], in_=vwp[c * P:(c + 1) * P, :])
            nc.vector.tensor_copy(out=vwp_sb, in_=vst)
            bpg_bc = wpool.tile([P, D], f32)
            nc.gpsimd.dma_start(out=bpg_bc, in_=bcast(bpg))
            gamma_bc = wpool.tile([P, D], f32)
            nc.gpsimd.dma_start(out=gamma_bc, in_=bcast(gamma))
            beta_bc = wpool.tile([P, D], f32)
            nc.gpsimd.dma_start(out=beta_bc, in_=bcast(beta))
            gate_sb = wpool.tile([P, 1], f32)
            nc.gpsimd.dma_start(out=gate_sb, in_=bcast(gate))

            pending = None  # (b, qb, wT, rgb_raw) awaiting proj
            for b in range(b_loc):
                po, pT, rT = pose_state
                uT = emit_ut(pT)
                if b + 1 < b_loc:
                    pose_state = emit_pose_dmas(b + 1)
                for qb in range(nqb):
                    rgb_raw = emit_rgb_raw(b, qb)
                    for j in range(tpb):
                        nc.vector.tensor_add(
                            out=rgb_raw[j], in0=rgb_raw[j], in1=bpg_bc)
                    attnT = emit_scores(uT, rT, qb)
                    if pending is not None:
                        emit_proj(*pending)
                    wT, csum = emit_wt(po, attnT)
                    pending = (b, qb, wT, csum, rgb_raw)
            emit_proj(*pending)

    nc.compile()
    return nc


def prep_inputs(inputs, b_loc=B_LOC, s=S, n_cores=N_CORES):
    """Host-side weight folding + padding + sharding -> per-core input maps."""
    import ml_dtypes
    f16 = ml_dtypes.float16 if hasattr(ml_dtypes, "float16") else np.float16

    g = {k: np.asarray(inputs[k], dtype=np.float64) for k in
         ("Wq", "bq", "Wk", "bk", "Wv", "bv", "Wp", "bp")}
    sc = 1.0 / np.sqrt(H)
    # pose-side feature d maps to padded slot: d for d<384, d+1 for d>=384
    # (slot ONES_COL=384 holds the ones column that yields colsum(attn))
    pslot = np.concatenate([np.arange(384), np.arange(385, D + 1)])
    X = np.zeros((DP, DP), np.float32)
    X[pslot, :D] = (g["Wk"] @ g["Wq"].T) * sc
    X[pslot, CK_COL] = (g["Wk"] @ g["bq"]) * sc
    VWp = np.zeros((DP, D), np.float32)
    VWp[pslot, :] = g["Wv"] @ g["Wp"]
    gate = np.asarray(inputs["gate"], dtype=np.float32)
    bpg = (gate[0] * (g["bp"] + g["bv"] @ g["Wp"])).astype(np.float32)

    rgb = np.asarray(inputs["rgb"], dtype=np.float32)
    pose = np.asarray(inputs["pose"], dtype=np.float32)
    nb = rgb.shape[0]
    rgbp = np.zeros((nb, s, DP), dtype=f16)
    rgbp[:, :, :D] = rgb.astype(f16)
    rgbp[:, :, CK_COL] = 1.0
    rgbpT = np.ascontiguousarray(rgbp.transpose(0, 2, 1))
    posep = np.zeros((nb, s, DP), dtype=f16)
    posep[:, :, pslot] = pose.astype(f16)
    posep[:, :, ONES_COL] = 1.0
    posepT = np.ascontiguousarray(posep.transpose(0, 2, 1))

    shared = {
        "xw": X.astype(f16),
        "vwp": VWp,
        "bpg": bpg,
        "ln_gamma": np.ascontiguousarray(inputs["ln_gamma"], dtype=np.float32),
        "ln_beta": np.ascontiguousarray(inputs["ln_beta"], dtype=np.float32),
        "gate": gate,
    }
    maps = []
    for i in range(n_cores):
        m = dict(shared)
        sl = slice(i * b_loc, (i + 1) * b_loc)
        m["rgb"] = np.ascontiguousarray(rgb[sl])
        m["rgbpT"] = np.ascontiguousarray(rgbpT[sl])
        m["posep"] = np.ascontiguousarray(posep[sl])
        m["posepT"] = np.ascontiguousarray(posepT[sl])
        maps.append(m)
    return maps


_CACHE = {}


def kernel(**inputs):
    from concourse.bass_utils import run_bass_kernel_spmd

    if "nc" not in _CACHE:
        _CACHE["nc"] = build_nc()
    nc = _CACHE["nc"]

    in_maps = prep_inputs(inputs)
    res = run_bass_kernel_spmd(nc, in_maps, list(range(N_CORES))).results
    return np.concatenate([res[i]["out"] for i in range(N_CORES)], axis=0)


# revision 32
# speedup vs baseline: 1.0317x; 1.0317x over previous
"""CrossModalFusion Trainium2 kernel (weight-folded G-route).

Reference computation (per batch b):
    q = rgb @ Wq + bq                 [S, H]
    k = pose @ Wk + bk                [S, H]
    v = pose @ Wv + bv                [S, H]
    attn = softmax(q @ k.T / sqrt(H)) [S, S]
    out  = attn @ v                   [S, H]
    proj = out @ Wp + bp              [S, D]
    x = rgb + gate * proj
    fused = LayerNorm(x) * gamma + beta

Algebraic restructure (weights folded on the HOST, once):
    X   = (Wk @ Wq.T) / sqrt(H)   [D, D]   so scoresT = (pose X^T?) ... precisely:
          scoresT[sk,sq] = sum_d' uT[d',sk] * rgbT[d',sq],  uT = X.T @ poseT
    c_k = (Wk @ bq) / sqrt(H)     folded as column D of X with a ones-column
          appended to rgb (terms depending only on sq cancel in softmax)
    VWp = Wv @ Wp                 [D, D]   proj = (attn @ pose) @ VWp
    bpg = gate * (bp + bv @ Wp)   added to rgb once per row tile
    colsum(attn) comes FREE from a ones-column in pose (partition 32 of the
          last d-chunk of wT = attn @ pose_aug)

Device work per batch (all matmuls, no PE transposes -- pose/rgb are
transposed by the DMA X-bar in fp16):
    uT[d',sk]  = X-chunks.T @ poseT        (64 MM of N=512)
    per 512-col query block:
      scoresT  = uT-chunks.T @ rgbT        (64 MM)  -> exp on ACT -> attnT fp16
      wT       = pose_aug-chunks.T @ attnT (64 MM)  [row 32 of chunk3 = colsum]
      proj     = wT-chunks.T @ VWp         (16 MM of N=400)
      residual + LayerNorm on DVE/ACT, store

Sharding: pure data-parallel over batch B=32 across 8 cores (4 each).
"""

import numpy as np

B, S, D, H = 32, 2048, 400, 512
DP = 512                 # padded feature dim (multiple of 128)
N_CORES = 8
B_LOC = B // N_CORES
LN_EPS = 1e-5
P = 128
QBLK = 512
ONES_COL = 384           # pose_aug ones column -> csum at partition 0 of chunk3
                         # (d-columns 384:400 shift up by one to 385:401)
CK_COL = 400             # X column holding Wk@bq; rgb_aug ones column

NDCH = DP // P           # 4 d-chunks


def build_nc(b_loc=B_LOC, s=S):
    import concourse.bass as bass
    import concourse.mybir as mybir
    import concourse.tile as tile
    from concourse import bacc

    def bcast(ap1d, p=P):
        return bass.AP(tensor=ap1d.tensor, offset=ap1d.offset,
                       ap=[[0, p]] + list(ap1d.ap))

    f32 = mybir.dt.float32
    f32r = mybir.dt.float32r
    f16 = mybir.dt.float16
    AF = mybir.ActivationFunctionType

    nt = s // P              # seq tiles (16)
    nqb = s // QBLK          # query blocks (4)
    tpb = QBLK // P          # row tiles per block (4)
    nnb = s // QBLK          # 512-wide column splits of s (4)

    nc = bacc.Bacc("TRN2", target_bir_lowering=False, debug=False,
                   num_swdge_queues=4)

    rgb = nc.dram_tensor("rgb", [b_loc, s, D], f32, kind="ExternalInput").ap()
    rgbpT = nc.dram_tensor("rgbpT", [b_loc, DP, s], f16, kind="ExternalInput").ap()
    posep = nc.dram_tensor("posep", [b_loc, s, DP], f16, kind="ExternalInput").ap()
    posepT = nc.dram_tensor("posepT", [b_loc, DP, s], f16, kind="ExternalInput").ap()
    xw = nc.dram_tensor("xw", [DP, DP], f16, kind="ExternalInput").ap()
    vwp = nc.dram_tensor("vwp", [DP, D], f32, kind="ExternalInput").ap()
    bpg = nc.dram_tensor("bpg", [D], f32, kind="ExternalInput").ap()
    gamma = nc.dram_tensor("ln_gamma", [D], f32, kind="ExternalInput").ap()
    beta = nc.dram_tensor("ln_beta", [D], f32, kind="ExternalInput").ap()
    gate = nc.dram_tensor("gate", [1], f32, kind="ExternalInput").ap()
    out = nc.dram_tensor("out", [b_loc, s, D], f32, kind="ExternalOutput").ap()

    from contextlib import ExitStack

    with tile.TileContext(nc) as tc:
        with ExitStack() as ctx:
            pool = lambda **kw: ctx.enter_context(tc.tile_pool(**kw))
            const = pool(name="const", bufs=1)
            wpool = pool(name="wpool", bufs=1)
            wstage = pool(name="wstage", bufs=1)
            ppool = pool(name="ppool", bufs=2)        # pose_aug natural (fp16)
            ptp = pool(name="ptp", bufs=2)            # poseT (fp16)
            rtp = pool(name="rtp", bufs=2)            # rgbT (fp16)
            utp = pool(name="utp", bufs=1)            # uT (fp16)
            atp = pool(name="atp", bufs=1)            # attnT (fp16)
            wtp = pool(name="wtp", bufs=2)            # wT (f32r)
            rpool = pool(name="rpool", bufs=2 * tpb)  # rgb raw (f32)
            ypool = pool(name="ypool", bufs=6)
            small = pool(name="small", bufs=6)
            ps_sc = pool(name="ps_sc", bufs=3, space="PSUM")
            ps_mm = pool(name="ps_mm", bufs=2, space="PSUM")
            ps_pj = pool(name="ps_pj", bufs=2, space="PSUM")
            ps_t1 = pool(name="ps_t1", bufs=1, space="PSUM")

            # ---- constants ----
            ones11 = const.tile([1, 1], f32)
            nc.vector.memset(ones11, 1.0)
            eps_sb = const.tile([P, 1], f32)
            nc.vector.memset(eps_sb, LN_EPS)
            warm_in = const.tile([P, P], f16)
            nc.vector.memset(warm_in, 0.0)

            def emit_pose_dmas(b):
                """Issue batch b's input DMAs, sliced so batch-0 compute can
                chase the transfers: poseT nb-slices first (uT), rgbT
                qb-slices on the scalar ring (scores), pose naturals last
                (wT needs them latest)."""
                pT = ptp.tile([P, NDCH, s], f16, tag="poseT")
                rT = rtp.tile([P, NDCH, s], f16, tag="rgbT")
                for nb in range(nnb):
                    for c in range(NDCH):
                        sl = slice(nb * QBLK, (nb + 1) * QBLK)
                        nc.sync.dma_start(
                            out=pT[:, c, sl],
                            in_=posepT[b, c * P:(c + 1) * P, sl])
                for nb in range(nnb):
                    for c in range(NDCH):
                        sl = slice(nb * QBLK, (nb + 1) * QBLK)
                        nc.sync.dma_start(
                            out=rT[:, c, sl],
                            in_=rgbpT[b, c * P:(c + 1) * P, sl])
                po = ppool.tile([P, nt, DP], f16, tag="pose")
                for t in range(nt):
                    nc.sync.dma_start(
                        out=po[:, t, :], in_=posep[b, t * P:(t + 1) * P, :])
                return po, pT, rT

            def emit_rgb_raw(b, qb):
                """Block qb's residual rgb rows (f32) + bpg pre-add."""
                tiles = []
                for j in range(tpb):
                    r0 = qb * QBLK + j * P
                    rr = rpool.tile([P, D], f32, tag="rraw")
                    nc.scalar.dma_start(out=rr, in_=rgb[b, r0:r0 + P, :])
                    tiles.append(rr)
                return tiles

            def emit_ut(pT):
                """uT[d',sk] = X-chunks.T @ poseT (fp16 out)."""
                uT = utp.tile([P, NDCH, s], f16, tag="uT")
                for nb in range(nnb):
                    for m in range(NDCH):
                        ps = ps_mm.tile([P, QBLK], f32, tag="mm")
                        for c in range(NDCH):
                            nc.tensor.matmul(
                                ps,
                                xw_sb[:, c, m * P:(m + 1) * P],
                                pT[:, c, nb * QBLK:(nb + 1) * QBLK],
                                start=(c == 0), stop=(c == NDCH - 1),
                            )
                        nc.vector.tensor_copy(
                            out=uT[:, m, nb * QBLK:(nb + 1) * QBLK], in_=ps)
                return uT

            def emit_scores(uT, rT, qb):
                """scoresT -> exp -> attnT (fp16, unnormalized)."""
                attnT = atp.tile([P, nt, QBLK], f16, tag="attnT")
                for t in range(nt):
                    ps = ps_sc.tile([P, QBLK], f32, tag="sc")
                    for c in range(NDCH):
                        nc.tensor.matmul(
                            ps,
                            uT[:, c, t * P:(t + 1) * P],
                            rT[:, c, qb * QBLK:(qb + 1) * QBLK],
                            start=(c == 0), stop=(c == NDCH - 1),
                        )
                    nc.scalar.activation(
                        out=attnT[:, t, :], in_=ps, func=AF.Exp)
                return attnT

            def emit_wt(po, attnT):
                """wT = pose_aug-chunks.T @ attnT; chunk3 row 0 = colsum.

                PSUM copies on ACT: during the wT phase ACT is idle (exps ran
                during scores), while the DVE FIFO holds the previous block's
                LayerNorm chain -- DVE copies here would stall the PE on PSUM
                bank recycling."""
                wT = wtp.tile([P, NDCH, QBLK], f32r, tag="wT")
                csum = wtp.tile([1, QBLK], f32, tag="csum")
                for m in range(NDCH):
                    ps = ps_mm.tile([P, QBLK], f32, tag="mm")
                    for t in range(nt):
                        nc.tensor.matmul(
                            ps,
                            po[:, t, m * P:(m + 1) * P],
                            attnT[:, t, :],
                            start=(t == 0), stop=(t == nt - 1),
                        )
                    nc.scalar.copy(out=wT[:, m, :], in_=ps)
                # row 0 of chunk3 = colsum; f32 copy for the f32 scatter matmul
                nc.scalar.copy(out=csum, in_=wT[0:1, 3, :])
                return wT, csum

            def emit_proj(b, qb, wT, csum, rgb_raw):
                """proj + gated residual (phase 1), then LayerNorm + store
                (phase 2).  Phased so the PSUM-freeing STT ops and the next
                block's work never queue behind the LN chain in the DVE
                FIFO."""
                q0 = qb * QBLK
                xs = []
                for j in range(tpb):
                    # csum chunk -> partitions via rank-1 matmul
                    pst = ps_t1.tile([P, 1], f32, tag="t1")
                    nc.tensor.matmul(
                        pst, csum[0:1, j * P:(j + 1) * P], ones11,
                        start=True, stop=True,
                    )
                    rec = small.tile([P, 1], f32, tag="rec")
                    nc.vector.reciprocal(out=rec, in_=pst)
                    gr = small.tile([P, 1], f32, tag="gr")
                    nc.vector.tensor_mul(out=gr, in0=rec, in1=gate_sb)

                    psp = ps_pj.tile([P, D], f32, tag="pj")
                    for c in range(NDCH):
                        # chunk3: K=17 includes csum partition 0 (vwp row is 0)
                        kc = P if c < 3 else (D - 3 * P + 1)
                        nc.tensor.matmul(
                            psp,
                            wT[:kc, c, j * P:(j + 1) * P],
                            vwp_sb[:kc, c, :],
                            start=(c == 0), stop=(c == NDCH - 1),
                        )
                    # x = gr * proj + (rgb + bpg)
                    x = ypool.tile([P, D], f32, tag="x")
                    nc.vector.scalar_tensor_tensor(
                        out=x, in0=psp, scalar=gr, in1=rgb_raw[j],
                        op0=mybir.AluOpType.mult, op1=mybir.AluOpType.add,
                    )
                    xs.append(x)
                for j, x in enumerate(xs):
                    stats = small.tile([P, 6], f32, tag="stats")
                    nc.vector.bn_stats(out=stats, in_=x)
                    mv = small.tile([P, 2], f32, tag="mv")
                    nc.vector.bn_aggr(out=mv, in_=stats)
                    sd = small.tile([P, 1], f32, tag="sd")
                    nc.scalar.activation(
                        out=sd, in_=mv[:, 1:2], func=AF.Sqrt, bias=eps_sb)
                    rstd = small.tile([P, 1], f32, tag="rstd")
                    nc.vector.reciprocal(out=rstd, in_=sd)
                    nc.vector.tensor_scalar(
                        out=x, in0=x, scalar1=mv[:, 0:1], scalar2=rstd,
                        op0=mybir.AluOpType.subtract, op1=mybir.AluOpType.mult,
                    )
                    nc.vector.tensor_mul(out=x, in0=x, in1=gamma_bc)
                    nc.vector.tensor_add(out=x, in0=x, in1=beta_bc)
                    nc.scalar.dma_start(
                        out=out[b, q0 + j * P:q0 + (j + 1) * P, :], in_=x)

            # ---- batch 0 input DMAs first (overlap with weight loads) ----
            pose_state = emit_pose_dmas(0)

            # ---- HAM warmup: keep PE busy while DMAs land ----
            for i in range(70):
                wps = ps_sc.tile([P, QBLK], f32, tag="sc")
                nc.tensor.matmul(
                    wps[:, :P], warm_in, warm_in, start=True, stop=True)

            # ---- weights ----
            xw_sb = wpool.tile([P, NDCH, DP], f16)
            for c in range(NDCH):
                nc.gpsimd.dma_start(
                    out=xw_sb[:, c, :], in_=xw[c * P:(c + 1) * P, :])
            vst = wstage.tile([P, NDCH, D], f32, tag="vst")
            vwp_sb = wpool.tile([P, NDCH, D], f32r)
            for c in range(NDCH):
                nc.gpsimd.dma_start(
                    out=vst[:, c, :], in_=vwp[c * P:(c + 1) * P, :])
            nc.vector.tensor_copy(out=vwp_sb, in_=vst)
            bpg_bc = wpool.tile([P, D], f32)
            nc.gpsimd.dma_start(out=bpg_bc, in_=bcast(bpg))
            gamma_bc = wpool.tile([P, D], f32)
            nc.gpsimd.dma_start(out=gamma_bc, in_=bcast(gamma))
            beta_bc = wpool.tile([P, D], f32)
            nc.gpsimd.dma_start(out=beta_bc, in_=bcast(beta))
            gate_sb = wpool.tile([P, 1], f32)
            nc.gpsimd.dma_start(out=gate_sb, in_=bcast(gate))

            pending = None  # (b, qb, wT, rgb_raw) awaiting proj
            for b in range(b_loc):
                po, pT, rT = pose_state
                uT = emit_ut(pT)
                if b + 1 < b_loc:
                    pose_state = emit_pose_dmas(b + 1)
                for qb in range(nqb):
                    rgb_raw = emit_rgb_raw(b, qb)
                    for j in range(tpb):
                        nc.vector.tensor_add(
                            out=rgb_raw[j], in0=rgb_raw[j], in1=bpg_bc)
                    attnT = emit_scores(uT, rT, qb)
                    if pending is not None:
                        emit_proj(*pending)
                    wT, csum = emit_wt(po, attnT)
                    pending = (b, qb, wT, csum, rgb_raw)
            emit_proj(*pending)

    nc.compile()
    return nc


def prep_inputs(inputs, b_loc=B_LOC, s=S, n_cores=N_CORES):
    """Host-side weight folding + padding + sharding -> per-core input maps."""
    import ml_dtypes
    f16 = ml_dtypes.float16 if hasattr(ml_dtypes, "float16") else np.float16

    g = {k: np.asarray(inputs[k], dtype=np.float64) for k in
         ("Wq", "bq", "Wk", "bk", "Wv", "bv", "Wp", "bp")}
    sc = 1.0 / np.sqrt(H)
    # pose-side feature d maps to padded slot: d for d<384, d+1 for d>=384
    # (slot ONES_COL=384 holds the ones column that yields colsum(attn))
    pslot = np.concatenate([np.arange(384), np.arange(385, D + 1)])
    X = np.zeros((DP, DP), np.float32)
    X[pslot, :D] = (g["Wk"] @ g["Wq"].T) * sc
    X[pslot, CK_COL] = (g["Wk"] @ g["bq"]) * sc
    VWp = np.zeros((DP, D), np.float32)
    VWp[pslot, :] = g["Wv"] @ g["Wp"]
    gate = np.asarray(inputs["gate"], dtype=np.float32)
    bpg = (gate[0] * (g["bp"] + g["bv"] @ g["Wp"])).astype(np.float32)

    rgb = np.asarray(inputs["rgb"], dtype=np.float32)
    pose = np.asarray(inputs["pose"], dtype=np.float32)
    nb = rgb.shape[0]
    rgbp = np.zeros((nb, s, DP), dtype=f16)
    rgbp[:, :, :D] = rgb.astype(f16)
    rgbp[:, :, CK_COL] = 1.0
    rgbpT = np.ascontiguousarray(rgbp.transpose(0, 2, 1))
    posep = np.zeros((nb, s, DP), dtype=f16)
    posep[:, :, pslot] = pose.astype(f16)
    posep[:, :, ONES_COL] = 1.0
    posepT = np.ascontiguousarray(posep.transpose(0, 2, 1))

    shared = {
        "xw": X.astype(f16),
        "vwp": VWp,
        "bpg": bpg,
        "ln_gamma": np.ascontiguousarray(inputs["ln_gamma"], dtype=np.float32),
        "ln_beta": np.ascontiguousarray(inputs["ln_beta"], dtype=np.float32),
        "gate": gate,
    }
    maps = []
    for i in range(n_cores):
        m = dict(shared)
        sl = slice(i * b_loc, (i + 1) * b_loc)
        m["rgb"] = np.ascontiguousarray(rgb[sl])
        m["rgbpT"] = np.ascontiguousarray(rgbpT[sl])
        m["posep"] = np.ascontiguousarray(posep[sl])
        m["posepT"] = np.ascontiguousarray(posepT[sl])
        maps.append(m)
    return maps


_CACHE = {}


def kernel(**inputs):
    from concourse.bass_utils import run_bass_kernel_spmd

    if "nc" not in _CACHE:
        _CACHE["nc"] = build_nc()
    nc = _CACHE["nc"]

    in_maps = prep_inputs(inputs)
    res = run_bass_kernel_spmd(nc, in_maps, list(range(N_CORES))).results
    return np.concatenate([res[i]["out"] for i in range(N_CORES)], axis=0)
